# revision 10
# baseline (speedup 1.0000x reference)
"""Diagonally-masked multi-head self-attention on 8 TRN2 NeuronCores.

Sharding (per the tensor/data-parallel hint, hardcoded):
  core c in 0..7 -> batch b = c // 4, head group g = c % 4 (4 heads each).
  Each core computes its batch's attention for its 4 heads plus the partial
  output projection (rows of Wo for its heads); the 4 partial outputs per
  batch are summed on the host (the "all-reduce").

Per-core kernel layout (all matmuls fp32r, fp32 accumulate):
  - inputs are pre-transposed/sliced on the host: xT [1024, 2048],
    wq/wk/wv [1024, 256] (columns of this head group), wo [256, 1024] (rows).
  - QT/KT [2*64, 2048] per head pair (head-dim on partitions) so that both
    the score matmuls (K=64, row-tiled 2-heads-concurrent) and the PV matmul
    (keys on partitions) need no transposes.
  - scores are built transposed, ST[k, q]; softmax denominator comes from a
    ones column appended to V (row 64 of the PV accumulator), so no
    partition reduction is ever needed.
  - diagonal mask = multiply exp(scores) by (1 - I) on the block where
    key-tile and query-chunk overlap.
"""

import numpy as np

import concourse.bass as bass
import concourse.mybir as mybir
import concourse.tile as tile
from concourse import bacc
from concourse.bass_utils import run_bass_kernel_spmd

B, L, DIM = 2, 2048, 1024
H, D = 16, 64
NCORES = 8
HPC = 4  # heads per core
GCOLS = HPC * D  # 256 weight cols per core
KCH = DIM // 128  # 8 contraction chunks for the projections
QC = L // 512  # 4 query chunks
JT = L // 128  # 16 key tiles
SCALE = 1.0 / 8.0  # 1/sqrt(D)

F32 = mybir.dt.float32
F32R = mybir.dt.float32r
EXP = mybir.ActivationFunctionType.Exp


_NC_CACHE = {}


def _build_nc():
    if "nc" in _NC_CACHE:
        return _NC_CACHE["nc"]

    nc = bacc.Bacc("TRN2", target_bir_lowering=False, debug=False, num_devices=NCORES)

    xT_d = nc.dram_tensor("xT", [DIM, L], F32R, kind="ExternalInput")
    wq_d = nc.dram_tensor("wq", [DIM, GCOLS], F32R, kind="ExternalInput")
    wk_d = nc.dram_tensor("wk", [DIM, GCOLS], F32R, kind="ExternalInput")
    wv_d = nc.dram_tensor("wv", [DIM, GCOLS], F32R, kind="ExternalInput")
    wo_d = nc.dram_tensor("wo", [GCOLS, DIM], F32R, kind="ExternalInput")
    out_d = nc.dram_tensor("out", [L, DIM], F32, kind="ExternalOutput")
    diag_d = nc.inline_tensor(
        np.ascontiguousarray((1.0 - np.eye(128)).astype(np.float32)), name="diagmask"
    )

    with tile.TileContext(nc) as tc:
        with (
            tc.tile_pool(name="singles", bufs=1) as singles,
            tc.tile_pool(name="big", bufs=8) as big,
            tc.tile_pool(name="otn", bufs=8) as otnp,
            tc.tile_pool(name="osb", bufs=2) as outp,
            tc.tile_pool(name="rd", bufs=2) as rdp,
            tc.tile_pool(name="rdb", bufs=2) as rdbp,
            tc.tile_pool(name="bp", bufs=2, space="PSUM") as bp,
            tc.tile_pool(name="otps", bufs=2, space="PSUM") as otp,
            tc.tile_pool(name="rdbps", bufs=2, space="PSUM") as rdbpsp,
        ):
            # ---- static loads -------------------------------------------
            wq_t = singles.tile([128, KCH, GCOLS], F32R, tag="wq")
            wk_t = singles.tile([128, KCH, GCOLS], F32R, tag="wk")
            wv_t = singles.tile([128, KCH, GCOLS], F32R, tag="wv")
            wo_t = singles.tile([64, HPC, DIM], F32R, tag="wo")
            diag_t = singles.tile([128, 128], F32, tag="diag")
            ones_t = singles.tile([128, 64], F32R, tag="ones")
            nc.vector.memset(ones_t[:].bitcast(F32), 1.0)
            vaug = singles.tile([128, JT, HPC, D + 1], F32R, tag="vaug")
            qt = [singles.tile([128, L], F32R, tag=f"qt{p}", name=f"qt{p}") for p in range(2)]
            kt = [singles.tile([128, L], F32R, tag=f"kt{p}", name=f"kt{p}") for p in range(2)]

            nc.sync.dma_start(out=wq_t, in_=wq_d[:].rearrange("(c p) n -> p c n", p=128))
            nc.sync.dma_start(out=wk_t, in_=wk_d[:].rearrange("(c p) n -> p c n", p=128))
            nc.sync.dma_start(out=wv_t, in_=wv_d[:].rearrange("(c p) n -> p c n", p=128))
            nc.sync.dma_start(out=wo_t, in_=wo_d[:].rearrange("(h p) n -> p h n", p=64))
            nc.sync.dma_start(out=diag_t, in_=diag_d[:])
            nc.vector.memset(vaug[:, :, :, D].bitcast(F32), 1.0)

            xt = []
            for k in range(KCH):
                xk = big.tile([128, L], F32R, tag="big", name=f"xt{k}")
                nc.sync.dma_start(out=xk, in_=xT_d[128 * k : 128 * (k + 1), :])
                xt.append(xk)

            # ---- phase 1a: QT / KT  (head-dim on partitions) ------------
            for pair in range(2):
                for wt, dst in ((wq_t, qt[pair]), (wk_t, kt[pair])):
                    for c4 in range(QC):
                        ps = bp.tile([128, 512], F32, tag="bp")
                        for k in range(KCH):
                            nc.tensor.matmul(
                                out=ps,
                                lhsT=(wt[:, k, 128 * pair : 128 * (pair + 1)]),
                                rhs=(xt[k][:, 512 * c4 : 512 * (c4 + 1)]),
                                start=(k == 0),
                                stop=(k == KCH - 1),
                            )
                        nc.vector.tensor_copy(
                            out=dst[:, 512 * c4 : 512 * (c4 + 1)], in_=ps
                        )

            # ---- phase 1b: V (natural layout, keys on partitions) -------
            for t in range(JT):
                ps = bp.tile([128, GCOLS], F32, tag="bp")
                for k in range(KCH):
                    nc.tensor.matmul(
                        out=ps,
                        lhsT=(xt[k][:, 128 * t : 128 * (t + 1)]),
                        rhs=(wv_t[:, k, :]),
                        start=(k == 0),
                        stop=(k == KCH - 1),
                    )
                for h in range(HPC):
                    nc.vector.tensor_copy(
                        out=vaug[:, t, h, 0:D], in_=ps[:, D * h : D * (h + 1)]
                    )

            # ---- phase 2/3: attention ------------------------------------
            otn = {}
            for c in range(QC):
                for pair in range(2):
                    ha, hb = 2 * pair, 2 * pair + 1
                    ot_a = otp.tile([D + 1, 512], F32, tag="ot")
                    ot_b = otp.tile([D + 1, 512], F32, tag="ot")
                    for j in range(JT):
                        st = bp.tile([128, 1024], F32, tag="bp")
                        # scores (transposed): ST[k-tile, q-chunk], both heads
                        # of the pair run concurrently via row tiling (K=64).
                        nc.tensor.matmul(
                            out=st[:, 0:512],
                            lhsT=(kt[pair][0:64, 128 * j : 128 * (j + 1)]),
                            rhs=(qt[pair][0:64, 512 * c : 512 * (c + 1)]),
                            start=True,
                            stop=True,
                        )
                        nc.tensor.matmul(
                            out=st[:, 512:1024],
                            lhsT=(kt[pair][64:128, 128 * j : 128 * (j + 1)]),
                            rhs=(qt[pair][64:128, 512 * c : 512 * (c + 1)]),
                            start=True,
                            stop=True,
                        )
                        et = big.tile([128, 1024], F32R, tag="big")
                        nc.scalar.activation(out=et, in_=st, func=EXP, scale=SCALE)
                        if 4 * c <= j < 4 * (c + 1):
                            off = 128 * (j - 4 * c)
                            nc.vector.tensor_mul(
                                out=et[:, off : off + 128],
                                in0=et[:, off : off + 128],
                                in1=diag_t,
                            )
                            nc.vector.tensor_mul(
                                out=et[:, 512 + off : 512 + off + 128],
                                in0=et[:, 512 + off : 512 + off + 128],
                                in1=diag_t,
                            )
                        # PV (+ denominator in row 64 via the ones column)
                        nc.tensor.matmul(
                            out=ot_a,
                            lhsT=(vaug[:, j, ha, :]),
                            rhs=(et[:, 0:512]),
                            start=(j == 0),
                            stop=(j == JT - 1),
                        )
                        nc.tensor.matmul(
                            out=ot_b,
                            lhsT=(vaug[:, j, hb, :]),
                            rhs=(et[:, 512:1024]),
                            start=(j == 0),
                            stop=(j == JT - 1),
                        )
                    for h, ot in ((ha, ot_a), (hb, ot_b)):
                        rd = rdp.tile([D + 1, 512], F32R, tag="rd")
                        with nc.allow_low_precision(reason="1/D rounded to fp32r"):
                            nc.vector.reciprocal(
                                out=rd[D : D + 1, :], in_=ot[D : D + 1, :]
                            )
                        # broadcast 1/D (partition 64) to 64 partitions via PE
                        rdb_ps = rdbpsp.tile([D, 512], F32, tag="rdbps")
                        nc.tensor.matmul(
                            out=rdb_ps,
                            lhsT=ones_t[D : D + 1, :],
                            rhs=rd[D : D + 1, :],
                            start=True,
                            stop=True,
                        )
                        rdb = rdbp.tile([D, 512], F32, tag="rdb")
                        nc.vector.tensor_copy(out=rdb, in_=rdb_ps)
                        otn_t = otnp.tile([D, 512], F32R, tag="otn")
                        nc.vector.tensor_mul(out=otn_t, in0=ot[0:D, :], in1=rdb[:])
                        otn[(h, c)] = otn_t

                # ---- phase 4: output projection for this query chunk -----
                for tt in range(4):
                    t = 4 * c + tt
                    onp = bp.tile([128, 1024], F32, tag="bp")
                    for h in range(HPC):
                        lhs = otn[(h, c)][:, 128 * tt : 128 * (tt + 1)]
                        nc.tensor.matmul(
                            out=onp[:, 0:512],
                            lhsT=(lhs),
                            rhs=(wo_t[:, h, 0:512]),
                            start=(h == 0),
                            stop=(h == HPC - 1),
                        )
                        nc.tensor.matmul(
                            out=onp[:, 512:1024],
                            lhsT=(lhs),
                            rhs=(wo_t[:, h, 512:1024]),
                            start=(h == 0),
                            stop=(h == HPC - 1),
                        )
                    osb = outp.tile([128, 1024], F32, tag="osb")
                    nc.vector.tensor_copy(out=osb, in_=onp)
                    nc.sync.dma_start(
                        out=out_d[128 * t : 128 * (t + 1), :], in_=osb
                    )

    nc.compile()
    _NC_CACHE["nc"] = nc
    return nc


def make_in_maps(x, Wq, Wk, Wv, Wo):
    x = np.asarray(x, dtype=np.float32)
    Wq = np.asarray(Wq, dtype=np.float32)
    Wk = np.asarray(Wk, dtype=np.float32)
    Wv = np.asarray(Wv, dtype=np.float32)
    Wo = np.asarray(Wo, dtype=np.float32)
    in_maps = []
    for core in range(NCORES):
        b, g = core // HPC, core % HPC
        cs = slice(GCOLS * g, GCOLS * (g + 1))
        in_maps.append(
            {
                "xT": np.ascontiguousarray(x[b].T),
                "wq": np.ascontiguousarray(Wq[:, cs]),
                "wk": np.ascontiguousarray(Wk[:, cs]),
                "wv": np.ascontiguousarray(Wv[:, cs]),
                "wo": np.ascontiguousarray(Wo[cs, :]),
            }
        )
    return in_maps


def combine_outputs(results):
    out = np.zeros((B, L, DIM), dtype=np.float32)
    for core in range(NCORES):
        out[core // HPC] += results[core]["out"]
    return out


def kernel(x, Wq, Wk, Wv, Wo):
    nc = _build_nc()
    in_maps = make_in_maps(x, Wq, Wk, Wv, Wo)
    res = run_bass_kernel_spmd(nc, in_maps, core_ids=list(range(NCORES)))
    return combine_outputs(res.results)


# revision 22
# speedup vs baseline: 1.0067x; 1.0067x over previous
"""Diagonally-masked multi-head self-attention on 8 TRN2 NeuronCores.

Sharding (per the tensor/data-parallel hint, hardcoded):
  core c in 0..7 -> batch b = c // 4, head group g = c % 4 (4 heads each).
  Each core computes its batch's attention for its 4 heads plus the partial
  output projection (rows of Wo for its heads); the 4 partial outputs per
  batch are summed on the host (the "all-reduce").

Per-core kernel layout (all matmuls fp32r, fp32 accumulate):
  - inputs are pre-transposed/sliced on the host: xT [1024, 2048],
    wq/wk/wv [1024, 256] (columns of this head group), wo [256, 1024] (rows).
  - QT/KT [2*64, 2048] per head pair (head-dim on partitions) so that both
    the score matmuls (K=64, row-tiled 2-heads-concurrent) and the PV matmul
    (keys on partitions) need no transposes.
  - scores are built transposed, ST[k, q]; softmax denominator comes from a
    ones column appended to V (row 64 of the PV accumulator), so no
    partition reduction is ever needed.
  - diagonal mask = multiply exp(scores) by (1 - I) on the block where
    key-tile and query-chunk overlap.
  - projections for query chunk c are emitted after attention chunk c+1 so
    the scheduler slots them into PE gaps while ACT (the bottleneck) runs.
"""

import numpy as np

import concourse.bass as bass
import concourse.mybir as mybir
import concourse.tile as tile
from concourse import bacc
from concourse.bass_utils import run_bass_kernel_spmd

B, L, DIM = 2, 2048, 1024
H, D = 16, 64
NCORES = 8
HPC = 4  # heads per core
GCOLS = HPC * D  # 256 weight cols per core
KCH = DIM // 128  # 8 contraction chunks for the projections
QC = L // 512  # 4 query chunks
JT = L // 128  # 16 key tiles
SCALE = 1.0 / 8.0  # 1/sqrt(D)

F32 = mybir.dt.float32
F32R = mybir.dt.float32r
EXP = mybir.ActivationFunctionType.Exp


_NC_CACHE = {}


def _build_nc():
    if "nc" in _NC_CACHE:
        return _NC_CACHE["nc"]

    nc = bacc.Bacc("TRN2", target_bir_lowering=False, debug=False, num_devices=NCORES)

    xT_d = nc.dram_tensor("xT", [DIM, L], F32R, kind="ExternalInput")
    wq_d = nc.dram_tensor("wq", [DIM, GCOLS], F32R, kind="ExternalInput")
    wk_d = nc.dram_tensor("wk", [DIM, GCOLS], F32R, kind="ExternalInput")
    wv_d = nc.dram_tensor("wv", [DIM, GCOLS], F32R, kind="ExternalInput")
    wo_d = nc.dram_tensor("wo", [GCOLS, DIM], F32R, kind="ExternalInput")
    out_d = nc.dram_tensor("out", [L, DIM], F32, kind="ExternalOutput")
    diag_d = nc.inline_tensor(
        np.ascontiguousarray((1.0 - np.eye(128)).astype(np.float32)), name="diagmask"
    )

    with tile.TileContext(nc) as tc:
        with (
            tc.tile_pool(name="singles", bufs=1) as singles,
            tc.tile_pool(name="big", bufs=8) as big,
            tc.tile_pool(name="etp", bufs=3) as etp,
            tc.tile_pool(name="otn", bufs=6) as otnp,
            tc.tile_pool(name="tmpp", bufs=2) as tmpp,
            tc.tile_pool(name="osb", bufs=2) as outp,
            tc.tile_pool(name="rd", bufs=3) as rdp,
            tc.tile_pool(name="bp", bufs=2, space="PSUM") as bp,
            tc.tile_pool(name="otps", bufs=2, space="PSUM") as otp,
            tc.tile_pool(name="smp", bufs=2, space="PSUM") as smp,
        ):
            # ---- static loads -------------------------------------------
            wq_t = singles.tile([128, KCH, GCOLS], F32R, tag="wq")
            wk_t = singles.tile([128, KCH, GCOLS], F32R, tag="wk")
            wv_t = singles.tile([128, KCH, GCOLS], F32R, tag="wv")
            wo_t = singles.tile([128, 2, DIM], F32R, tag="wo")
            diag_t = singles.tile([128, 128], F32, tag="diag")
            ones_t = singles.tile([128, 64], F32R, tag="ones")
            vaug = singles.tile([128, JT, HPC, D + 1], F32R, tag="vaug")
            qt = [singles.tile([128, L], F32R, tag=f"qt{p}", name=f"qt{p}") for p in range(2)]
            kt = [singles.tile([128, L], F32R, tag=f"kt{p}", name=f"kt{p}") for p in range(2)]

            nc.sync.dma_start(out=wq_t, in_=wq_d[:].rearrange("(c p) n -> p c n", p=128))
            nc.sync.dma_start(out=wk_t, in_=wk_d[:].rearrange("(c p) n -> p c n", p=128))
            xt = []
            for k in range(KCH):
                xk = big.tile([128, L], F32R, tag="big", name=f"xt{k}")
                nc.sync.dma_start(out=xk, in_=xT_d[128 * k : 128 * (k + 1), :])
                xt.append(xk)
            nc.sync.dma_start(out=wv_t, in_=wv_d[:].rearrange("(c p) n -> p c n", p=128))
            nc.sync.dma_start(out=wo_t, in_=wo_d[:].rearrange("(g p) n -> p g n", p=128))
            nc.sync.dma_start(out=diag_t, in_=diag_d[:])
            nc.vector.memset(ones_t[:].bitcast(F32), 1.0)
            nc.vector.memset(vaug[:, :, :, D].bitcast(F32), 1.0)

            def qk_group(pair, qk, c4):
                """One [128, 512] accumulation group of QT or KT."""
                wt, dst = ((wq_t, qt[pair]), (wk_t, kt[pair]))[qk]
                nm = f"ps{'qk'[qk]}{pair}_{c4}"
                ps = smp.tile([128, 512], F32, tag="sm", name=nm)
                for k in range(KCH):
                    nc.tensor.matmul(
                        out=ps,
                        lhsT=wt[:, k, 128 * pair : 128 * (pair + 1)],
                        rhs=xt[k][:, 512 * c4 : 512 * (c4 + 1)],
                        start=(k == 0),
                        stop=(k == KCH - 1),
                    )
                nc.vector.tensor_copy(out=dst[:, 512 * c4 : 512 * (c4 + 1)], in_=ps)

            def v_group(t):
                ps = smp.tile([128, GCOLS], F32, tag="sm", name=f"psv{t}")
                for k in range(KCH):
                    nc.tensor.matmul(
                        out=ps,
                        lhsT=xt[k][:, 128 * t : 128 * (t + 1)],
                        rhs=wv_t[:, k, :],
                        start=(k == 0),
                        stop=(k == KCH - 1),
                    )
                for h in range(HPC):
                    nc.vector.tensor_copy(
                        out=vaug[:, t, h, 0:D], in_=ps[:, D * h : D * (h + 1)]
                    )

            otn = {}

            def attn(c, pair, pre_av=None, extras=None, stride=1):
                ha, hb = 2 * pair, 2 * pair + 1
                ot_a = otp.tile([D + 1, 512], F32, tag="ot", name=f"ota{c}_{pair}")
                ot_b = otp.tile([D + 1, 512], F32, tag="ot", name=f"otb{c}_{pair}")
                for j in range(JT):
                    st = bp.tile([128, 1024], F32, tag="bp", name=f"st{c}_{pair}_{j}")
                    # scores (transposed): ST[k-tile, q-chunk]; the two heads
                    # of the pair run concurrently via row tiling.
                    nc.tensor.matmul(
                        out=st[:, 0:512],
                        lhsT=kt[pair][0:64, 128 * j : 128 * (j + 1)],
                        rhs=qt[pair][0:64, 512 * c : 512 * (c + 1)],
                        start=True,
                        stop=True,
                    )
                    nc.tensor.matmul(
                        out=st[:, 512:1024],
                        lhsT=kt[pair][64:128, 128 * j : 128 * (j + 1)],
                        rhs=qt[pair][64:128, 512 * c : 512 * (c + 1)],
                        start=True,
                        stop=True,
                    )
                    et = etp.tile([128, 1024], F32R, tag="et", name=f"et{c}_{pair}_{j}")
                    nc.scalar.activation(out=et, in_=st, func=EXP, scale=SCALE)
                    if 4 * c <= j < 4 * (c + 1):
                        off = 128 * (j - 4 * c)
                        nc.vector.tensor_mul(
                            out=et[:, off : off + 128],
                            in0=et[:, off : off + 128],
                            in1=diag_t,
                        )
                        nc.vector.tensor_mul(
                            out=et[:, 512 + off : 512 + off + 128],
                            in0=et[:, 512 + off : 512 + off + 128],
                            in1=diag_t,
                        )
                    if pre_av is not None:
                        pre_av(j)
                    if extras and j % stride == stride - 1:
                        extras.pop(0)()
                    # PV (+ denominator in row 64 via the ones column)
                    nc.tensor.matmul(
                        out=ot_a,
                        lhsT=vaug[:, j, ha, :],
                        rhs=et[:, 0:512],
                        start=(j == 0),
                        stop=(j == JT - 1),
                    )
                    nc.tensor.matmul(
                        out=ot_b,
                        lhsT=vaug[:, j, hb, :],
                        rhs=et[:, 512:1024],
                        start=(j == 0),
                        stop=(j == JT - 1),
                    )
                def norm_half(h, ot, top):
                    def run():
                        rd = rdp.tile([D + 1, 512], F32R, tag="rd", name=f"rd{c}_{h}")
                        with nc.allow_low_precision(reason="1/D rounded to fp32r"):
                            nc.vector.reciprocal(
                                out=rd[D : D + 1, :], in_=ot[D : D + 1, :]
                            )
                        # broadcast 1/D (partition 64) to 64 partitions via PE
                        rdb_ps = smp.tile([D, 512], F32, tag="sm", name=f"rdps{c}_{h}")
                        nc.tensor.matmul(
                            out=rdb_ps,
                            lhsT=ones_t[D : D + 1, :],
                            rhs=rd[D : D + 1, :],
                            start=True,
                            stop=True,
                        )
                        rdb = rdp.tile([D, 512], F32, tag="rd", name=f"rdb{c}_{h}")
                        nc.vector.tensor_copy(out=rdb, in_=rdb_ps)
                        if top:
                            # heads 0/2 land on partitions 0-63 of the paired tile
                            otn2 = otnp.tile(
                                [128, 512], F32R, tag="otn", name=f"otn{c}_{pair}"
                            )
                            otn[(pair, c)] = otn2
                            nc.vector.tensor_mul(
                                out=otn2[0:D, :], in0=ot[0:D, :], in1=rdb[:]
                            )
                        else:
                            # heads 1/3: normalize then DMA-shift to partitions 64-127
                            tmp = tmpp.tile([D, 512], F32R, tag="tmp", name=f"otmp{c}_{pair}")
                            nc.vector.tensor_mul(out=tmp, in0=ot[0:D, :], in1=rdb[:])
                            nc.sync.dma_start(out=otn[(pair, c)][D : 2 * D, :], in_=tmp)

                    return run

                return [norm_half(ha, ot_a, True), norm_half(hb, ot_b, False)]

            def proj_group(c, tt, half):
                t = 4 * c + tt
                onp = smp.tile([128, 512], F32, tag="sm", name=f"onp{t}_{half}")
                for g in range(2):
                    nc.tensor.matmul(
                        out=onp,
                        lhsT=otn[(g, c)][:, 128 * tt : 128 * (tt + 1)],
                        rhs=wo_t[:, g, 512 * half : 512 * (half + 1)],
                        start=(g == 0),
                        stop=(g == 1),
                    )
                osb = outp.tile([128, 512], F32, tag="osb", name=f"osb{t}_{half}")
                nc.vector.tensor_copy(out=osb, in_=onp)
                nc.sync.dma_start(
                    out=out_d[128 * t : 128 * (t + 1), 512 * half : 512 * (half + 1)],
                    in_=osb,
                )

            def proj_thunks(c):
                return [
                    (lambda tt=tt, half=half: proj_group(c, tt, half))
                    for tt in range(4)
                    for half in range(2)
                ]

            def qk_thunk(pair, qk, c4):
                return lambda: qk_group(pair, qk, c4)

            # ---- emission order (priority): get ACT (exp) started ASAP,
            # then feed PE filler work (pair-1 QK projections, per-chunk
            # normalization, output projections) into the attention loops
            # at a rate that keeps ACT (the bottleneck engine) from starving.
            for c4 in range(QC):
                qk_group(0, 1, c4)  # KT pair 0
            qk_group(0, 0, 0)  # QT pair 0, chunk 0
            # V tiles are produced just-in-time ahead of each PV matmul
            n00 = attn(0, 0, pre_av=v_group)
            qk_group(0, 0, 1)
            n10 = attn(1, 0, extras=n00 + [qk_thunk(0, 0, 2), qk_thunk(0, 0, 3),
                                           qk_thunk(1, 1, 0), qk_thunk(1, 1, 1)],
                       stride=2)
            n20 = attn(2, 0, extras=n10 + [qk_thunk(1, 1, 2), qk_thunk(1, 1, 3),
                                           qk_thunk(1, 0, 0), qk_thunk(1, 0, 1)],
                       stride=2)
            n30 = attn(3, 0, extras=n20 + [qk_thunk(1, 0, 2), qk_thunk(1, 0, 3)],
                       stride=2)
            n01 = attn(0, 1, extras=n30, stride=2)
            n11 = attn(1, 1, extras=n01 + proj_thunks(0), stride=1)
            n21 = attn(2, 1, extras=n11 + proj_thunks(1), stride=1)
            n31 = attn(3, 1, extras=n21 + proj_thunks(2), stride=1)
            for th in n31 + proj_thunks(3):
                th()

    nc.compile()
    _NC_CACHE["nc"] = nc
    return nc


def make_in_maps(x, Wq, Wk, Wv, Wo):
    x = np.asarray(x, dtype=np.float32)
    Wq = np.asarray(Wq, dtype=np.float32)
    Wk = np.asarray(Wk, dtype=np.float32)
    Wv = np.asarray(Wv, dtype=np.float32)
    Wo = np.asarray(Wo, dtype=np.float32)
    in_maps = []
    for core in range(NCORES):
        b, g = core // HPC, core % HPC
        cs = slice(GCOLS * g, GCOLS * (g + 1))
        in_maps.append(
            {
                "xT": np.ascontiguousarray(x[b].T),
                "wq": np.ascontiguousarray(Wq[:, cs]),
                "wk": np.ascontiguousarray(Wk[:, cs]),
                "wv": np.ascontiguousarray(Wv[:, cs]),
                "wo": np.ascontiguousarray(Wo[cs, :]),
            }
        )
    return in_maps


def combine_outputs(results):
    out = np.zeros((B, L, DIM), dtype=np.float32)
    for core in range(NCORES):
        out[core // HPC] += results[core]["out"]
    return out


def kernel(x, Wq, Wk, Wv, Wo):
    nc = _build_nc()
    in_maps = make_in_maps(x, Wq, Wk, Wv, Wo)
    res = run_bass_kernel_spmd(nc, in_maps, core_ids=list(range(NCORES)))
    return combine_outputs(res.results)
